# revision 11
# baseline (speedup 1.0000x reference)
"""Trainium2 Bass kernel for nn_AugementationAttention.

Reference computation (per batch b of 16, N=512, D=4096, NH=16, DK=256):
    q = x @ Wq.T, k = x @ Wk.T, v = x @ Wv.T          (per-head dk=256)
    ep = aug @ Wp.T + bp                               (per-head n=512 == 2*dk)
    dist = softmax(q k^T / sqrt(dk) + ep, axis=-1)
    out  = dist @ v                                    -> (b, n, d)

Sharding: data-parallel over batch, 2 batch elements per core on 8 cores.

Per-core kernel structure (single fused pass over heads, batches paired):
  - bf16 everywhere on the projection matmuls (fp8 DoubleRow was measured
    in simulation at 7e-2 absmax rel err vs the 2e-2 gate -- dead), so the
    kernel is tensor-engine bound at the 78.6 TFLOP/s bf16 roofline:
    180.6 GFLOP/core -> 2294 us of pure column streaming.  Everything else
    here exists to keep the PE at that rate from the first microsecond:
  - Warmup: ~72 junk matmuls on a memset tile run during the initial DMA
    shadow so the HAM clock gate reaches K=8/8 (2.4 GHz) before real work
    and never re-throttles.
  - Head 0 interleaves the Q and K projections per k-tile (8 psum banks),
    halving its x-arrival bandwidth demand; heads 1+ run phase-by-phase
    from resident activations.  Its critical stream is choreographed
    across the three DMA queues: first-needed chunks land in parallel,
    the next wq/wk groups are prestaged (dedicated pool tag so triggers
    never stall on recycle), x streams in k-tile pairs in consumption
    order on the fast SWDGE queue, and the aug bulk is gated behind the
    q/k copies so it cannot starve the ~40 GB/s HWDGE weight rings.
  - Scores stay in the transposed [key, query] layout; softmax
    normalization over the partition (key) axis uses a ones-column
    appended to V; bp folds into ScalarE's exp bias.  exp(S) and V are
    stored bf16, so the A@V stationary loads run at bf16 LDWEIGHTS speed
    and stay hidden.  The A@V stage of head h is deferred past head
    h+1's Q projection so the PE never waits on ScalarE's exp.
  - Outputs store per-qt (128 KB DMAs) round-robin over all three rings,
    so the tail after the last matmul is short.

Measured: 2.360 ms HW exec (baseline 2.399 ms), rel err 4.6e-3, PE busy
99.4% of span with the 2294 us column stream accounting for 98.5% of
busy.  Runs land at 2.84 ms when the chip drops to 2.0 GHz (P0 power
state) -- that is environmental, not kernel-dependent.
"""

import sys

sys.path.insert(0, "/opt/trn_rl_repo")

import numpy as np
import ml_dtypes

import concourse.bacc as bacc
import concourse.mybir as mybir
import concourse.tile as tile
from concourse.bass_utils import run_bass_kernel_spmd

F32 = mybir.dt.float32
F32R = mybir.dt.float32r
BF16 = mybir.dt.bfloat16

B, N, D, NH, DK = 16, 512, 4096, 16, 256
NCORES = 8
BL = B // NCORES  # batch elements per core
NB = BL * N  # 1024: both batch elements side by side on the free dim
KT = D // 128  # 32 k-tiles of the contraction dim
G = 4  # k-tiles per weight DMA chunk
NG = KT // G  # chunks per head per projection
SCALE = 1.0 / np.sqrt(DK)


def _build_program():
    nc = bacc.Bacc(
        "TRN2",
        target_bir_lowering=False,
        debug=False,
        enable_asserts=False,
        num_devices=NCORES,
    )

    xt = nc.dram_tensor("xt", [128, KT, NB], BF16, kind="ExternalInput")
    at = nc.dram_tensor("at", [128, KT, NB], BF16, kind="ExternalInput")
    # w*[h, g, p, G*256]: per chunk g, k-tile j in chunk, 256 dout cols
    wq = nc.dram_tensor("wq", [NH, NG, 128, G * 256], BF16, kind="ExternalInput")
    wk = nc.dram_tensor("wk", [NH, NG, 128, G * 256], BF16, kind="ExternalInput")
    wv = nc.dram_tensor("wv", [NH, NG, 128, G * 256], BF16, kind="ExternalInput")
    # wp packed as 32 pseudo-heads of 256 jrow cols: index h*2 + jt_half
    wp = nc.dram_tensor("wp", [2 * NH, NG, 128, G * 256], BF16, kind="ExternalInput")
    bias = nc.dram_tensor("bias", [128, 64], F32, kind="ExternalInput")
    # bf16 output: halves the final store drain (the kernel tail) and the
    # mid-run store traffic; host upcasts.  Costs ~2e-3 absmax rel err.
    out = nc.dram_tensor("out", [BL, N, D], BF16, kind="ExternalOutput")

    with tile.TileContext(nc) as tc:
        with (
            tc.tile_pool(name="const", bufs=1) as const_pool,
            tc.tile_pool(name="acts", bufs=1) as act_pool,
            tc.tile_pool(name="wgt", bufs=4) as w_pool,
            tc.tile_pool(name="qk", bufs=2) as qk_pool,
            tc.tile_pool(name="vv", bufs=2) as v_pool,
            tc.tile_pool(name="ee", bufs=2) as e_pool,
            tc.tile_pool(name="oo", bufs=3) as o_pool,
            tc.tile_pool(name="rr", bufs=8) as r_pool,
            tc.tile_pool(name="psp", bufs=4, space="PSUM") as ps_a,
            tc.tile_pool(name="pss", bufs=4, space="PSUM") as ps_b,
        ):
            bias_sb = const_pool.tile([128, 64], F32)

            # HAM warmup: keep the PE busy (~3.5us) under the initial DMA
            # shadow so the clock gate is at 8/8 when real matmuls start.
            junk_sb = const_pool.tile([128, 128], BF16, name="junk_sb")
            nc.vector.memset(junk_sb[:], 0.0)
            warm_ps = ps_b.tile([128, N], F32, tag="ss", name="warm_ps")
            for _ in range(72):
                nc.tensor.matmul(
                    warm_ps[:, 0:128],
                    junk_sb[:, 0:128],
                    junk_sb[:, 0:128],
                    start=True,
                    stop=True,
                )

            # resident activations, streamed in consumption order on the two
            # SWDGE rings; the sync/scalar HWDGE rings carry only weights so
            # the first-head critical path is never queued behind bulk.
            xt_sb = act_pool.tile([128, KT, NB], BF16, tag="xt", name="xt_sb")
            at_sb = act_pool.tile([128, KT, NB], BF16, tag="at", name="at_sb")
            # First-needed chunks land in parallel on all three queues; the
            # next three wq/wk groups ride the fast SWDGE queue ahead of the
            # x bulk (dedicated 8-buf tag -- triggers never stall on pool
            # recycle), leaving the slow HWDGE rings only g4..g7 of head 0.
            # (A finer-grained first wave was tried and regressed: the DMA
            # rings are cold (~26 GB/s) for the first ~10us, so starting the
            # stream earlier just moves the idle into heads 0's body.)
            w0_pre = []
            for g in range(4):
                wtq0 = w_pool.tile([128, G, 256], BF16, tag="w0", bufs=8, name="wq0p")
                wtk0 = w_pool.tile([128, G, 256], BF16, tag="w0", bufs=8, name="wk0p")
                if g == 0:
                    # three queues in parallel for the three first-needed chunks
                    # (wk g0 split in halves: its kt0-1 stationaries gate the
                    # K matmuls right after the first Q matmuls)
                    nc.sync.dma_start(out=wtq0[:], in_=wq[0, g])
                    nc.scalar.dma_start(out=xt_sb[:, 0:2, :], in_=xt[:, 0:2, :])
                    nc.gpsimd.dma_start(out=wtk0[:, 0:2, :], in_=wk[0, 0][:, 0:512])
                    nc.gpsimd.dma_start(out=wtk0[:, 2:4, :], in_=wk[0, 0][:, 512:1024])
                    nc.scalar.dma_start(out=bias_sb[:], in_=bias[:])
                elif g == 1:
                    # the HWDGE rings are free again by now; keep the SWDGE
                    # chain short so g2's x pairs aren't queued behind these
                    nc.sync.dma_start(out=wtq0[:], in_=wq[0, g])
                    nc.scalar.dma_start(out=wtk0[:], in_=wk[0, g])
                else:
                    nc.gpsimd.dma_start(out=wtq0[:], in_=wq[0, g])
                    nc.gpsimd.dma_start(out=wtk0[:], in_=wk[0, g])
                # x pairs for this g-group follow its weights in the FIFO
                # (kt0-1 already went out on the scalar ring above)
                if g > 0:
                    sl = slice(4 * g, 4 * g + 2)
                    nc.gpsimd.dma_start(out=xt_sb[:, sl, :], in_=xt[:, sl, :])
                sl = slice(4 * g + 2, 4 * g + 4)
                nc.gpsimd.dma_start(out=xt_sb[:, sl, :], in_=xt[:, sl, :])
                w0_pre.append((wtq0, wtk0))
            # rest of x in kt pairs, consumption order; kt21/kt25 are
            # carried by the HWDGE rings behind head-0's weight chunks (see
            # proj_qk_head0) to relieve the SWDGE queue before kt26.
            for i in range(8, 16):
                k0 = 2 * i
                if k0 in (20, 24):
                    nc.gpsimd.dma_start(
                        out=xt_sb[:, k0 : k0 + 1, :], in_=xt[:, k0 : k0 + 1, :]
                    )
                else:
                    sl = slice(k0, k0 + 2)
                    nc.gpsimd.dma_start(out=xt_sb[:, sl, :], in_=xt[:, sl, :])

            def proj_qk(w_dram, h, name):
                """Q^T/K^T projection for both batches: psum[b][dt] [128, N]."""
                ps = [
                    [
                        ps_a.tile([128, N], F32, tag="pp", name=f"ps{name}{b}{i}")
                        for i in range(2)
                    ]
                    for b in range(BL)
                ]
                for g in range(NG):
                    wt = w_pool.tile([128, G, 256], BF16, tag="wqk", name=f"w{name}")
                    eng = nc.sync if g % 2 == 0 else nc.scalar
                    eng.dma_start(out=wt[:], in_=w_dram[h, g])
                    for j in range(G):
                        kt = g * G + j
                        st, sp = kt == 0, kt == KT - 1
                        for b in range(BL):
                            xk = xt_sb[:, kt, b * N : (b + 1) * N]
                            for dt in range(2):
                                nc.tensor.matmul(
                                    ps[b][dt][:],
                                    wt[:, j, dt * 128 : (dt + 1) * 128],
                                    xk,
                                    start=st,
                                    stop=sp,
                                )
                sbs = []
                for b in range(BL):
                    sb = qk_pool.tile(
                        [128, 2, N], BF16, tag=f"{name}t", name=f"{name}t_sb{b}"
                    )
                    for dt in range(2):
                        nc.vector.tensor_copy(sb[:, dt, :], ps[b][dt][:])
                    sbs.append(sb)
                return sbs

            def proj_qk_head0():
                """Head 0 only: Q and K interleaved per k-tile, so the x
                stream is consumed at half the bandwidth of a serial Q pass
                (x is still arriving from HBM while this runs)."""
                psq = [
                    [
                        ps_a.tile([128, N], F32, tag="pp", name=f"psq0{b}{i}")
                        for i in range(2)
                    ]
                    for b in range(BL)
                ]
                psk = [
                    [
                        ps_b.tile([128, N], F32, tag="ss", name=f"psk0{b}{i}")
                        for i in range(2)
                    ]
                    for b in range(BL)
                ]
                for g in range(NG):
                    # one weight stream per HWDGE ring: under DMA-arbiter
                    # pressure from the x bulk each ring sustains only
                    # ~40 GB/s, so wq and wk must not share a ring here
                    # (g<4 was prefetched above).
                    if g < 4:
                        wtq, wtk = w0_pre[g]
                    else:
                        wtq = w_pool.tile([128, G, 256], BF16, tag="wqk", name="wq0")
                        wtk = w_pool.tile([128, G, 256], BF16, tag="wqk", name="wk0")
                        nc.sync.dma_start(out=wtq[:], in_=wq[0, g])
                        nc.scalar.dma_start(out=wtk[:], in_=wk[0, g])
                        if g == NG - 1:
                            # ride behind the last weight chunks on the now
                            # idle HWDGE rings; needed at kt21/kt25
                            nc.sync.dma_start(
                                out=xt_sb[:, 21:22, :], in_=xt[:, 21:22, :]
                            )
                            nc.scalar.dma_start(
                                out=xt_sb[:, 25:26, :], in_=xt[:, 25:26, :]
                            )
                    for j in range(G):
                        kt = g * G + j
                        st, sp = kt == 0, kt == KT - 1
                        for b in range(BL):
                            xk = xt_sb[:, kt, b * N : (b + 1) * N]
                            for dt in range(2):
                                nc.tensor.matmul(
                                    psq[b][dt][:],
                                    wtq[:, j, dt * 128 : (dt + 1) * 128],
                                    xk,
                                    start=st,
                                    stop=sp,
                                )
                                nc.tensor.matmul(
                                    psk[b][dt][:],
                                    wtk[:, j, dt * 128 : (dt + 1) * 128],
                                    xk,
                                    start=st,
                                    stop=sp,
                                )
                qsbs, ksbs = [], []
                for b in range(BL):
                    qsb = qk_pool.tile([128, 2, N], BF16, tag="qt", name=f"qt_sb{b}")
                    ksb = qk_pool.tile([128, 2, N], BF16, tag="kt", name=f"kt_sb{b}")
                    for dt in range(2):
                        nc.vector.tensor_copy(qsb[:, dt, :], psq[b][dt][:])
                        nc.vector.tensor_copy(ksb[:, dt, :], psk[b][dt][:])
                    qsbs.append(qsb)
                    ksbs.append(ksb)
                # Gate the aug bulk behind the q copies with a REAL data
                # dependency (a memset has none and the Tile scheduler
                # hoists it): if aug starts any earlier, the SWDGE queue
                # races ahead and starves the HWDGE weight rings mid-phase.
                # The garbage cell is overwritten by the chunk-0 DMA (WAW).
                nc.vector.tensor_copy(at_sb[:, 0:1, 0:1], qsbs[0][:, 0:1, 0:1])
                for i in range(8):  # aug, needed from the first S phase
                    sl = slice(4 * i, 4 * i + 4)
                    nc.gpsimd.dma_start(out=at_sb[:, sl, :], in_=at[:, sl, :])
                return qsbs, ksbs

            def proj_v(h):
                """V projection (n-major) for both batches, ones col appended."""
                ps = [
                    [
                        ps_a.tile([128, N], F32, tag="pp", name=f"psv{b}{i}")
                        for i in range(2)
                    ]
                    for b in range(BL)
                ]
                for g in range(NG):
                    wt = w_pool.tile([128, G, 256], BF16, tag="wqk", name="wv")
                    eng = nc.sync if g % 2 == 0 else nc.scalar
                    eng.dma_start(out=wt[:], in_=wv[h, g])
                    for j in range(G):
                        kt = g * G + j
                        wvj = wt[:, j, :]
                        for b in range(BL):
                            for nt in range(4):
                                nc.tensor.matmul(
                                    ps[b][nt // 2][
                                        :, (nt % 2) * 256 : (nt % 2 + 1) * 256
                                    ],
                                    xt_sb[
                                        :, kt, b * N + nt * 128 : b * N + (nt + 1) * 128
                                    ],
                                    wvj,
                                    start=(kt == 0 and nt % 2 == 0),
                                    stop=(kt == KT - 1 and nt % 2 == 1),
                                )
                v_sbs = []
                for b in range(BL):
                    v_sb = v_pool.tile([128, 4, 264], BF16, tag="v", name=f"v_sb{b}")
                    for nt in range(4):
                        nc.vector.tensor_copy(
                            v_sb[:, nt, 0:256],
                            ps[b][nt // 2][:, (nt % 2) * 256 : (nt % 2 + 1) * 256],
                        )
                    nc.vector.memset(v_sb[:, :, 256:257], 1.0)
                    nc.vector.memset(v_sb[:, :, 257:258], 0.0)
                    v_sbs.append(v_sb)
                return v_sbs

            def attn_out(state):
                """Deferred A@V + normalize + store for the previous head."""
                if state is None:
                    return
                h, e_sbs, v_sbs = state
                for b in range(BL):
                    e_sb, v_sb = e_sbs[b], v_sbs[b]
                    ot_sb = o_pool.tile([128, 4, 256], BF16, tag="ot", name="ot_sb")
                    out_v = out[b].rearrange("(qt p) d -> p qt d", p=128)
                    for qt in range(4):
                        pso = ps_b.tile([128, 258], F32, tag="ss", name="pso")
                        for jt in range(4):
                            nc.tensor.matmul(
                                pso[:],
                                e_sb[:, jt, qt * 128 : (qt + 1) * 128],
                                v_sb[:, jt, 0:258],
                                start=(jt == 0),
                                stop=(jt == 3),
                            )
                        r = r_pool.tile([128, 1], F32, tag="r", name="r")
                        nc.vector.reciprocal(r[:], pso[:, 256:257])
                        nc.vector.tensor_scalar_mul(
                            ot_sb[:, qt, :], pso[:, 0:256], r[:]
                        )
                        # small per-qt stores, round-robin over all three rings
                        eng = (nc.sync, nc.scalar, nc.gpsimd)[(h * 8 + b * 4 + qt) % 3]
                        eng.dma_start(
                            out=out_v[:, qt, h * 256 : (h + 1) * 256],
                            in_=ot_sb[:, qt, :],
                        )

            pending = None
            for h in range(NH):
                if h == 0:
                    qt_sbs, kt_sbs = proj_qk_head0()
                else:
                    qt_sbs = proj_qk(wq, h, "q")
                    # previous head's A@V runs here: ScalarE has long finished
                    # the previous exp, and E/V tiles free up before this
                    # head's own S phase needs their pool slots.
                    attn_out(pending)
                    pending = None
                    kt_sbs = proj_qk(wk, h, "k")
                v_sbs = proj_v(h)

                # S^T[j, q] in two jt-halves: Wp-projection + QK^T, then exp
                # bf16 exp storage: the A@V stationary loads then run at
                # bf16 LDWEIGHTS speed (107ns, hidden) instead of fp32
                # (190ns, partially exposed)
                e_sbs = [
                    e_pool.tile([128, 4, N], BF16, tag="e", name=f"e_sb{b}")
                    for b in range(BL)
                ]
                for half in range(2):
                    pool = ps_b if half == 0 else ps_a
                    ptag = "ss" if half == 0 else "pp"
                    pss = [
                        [
                            pool.tile([128, N], F32, tag=ptag, name=f"pss{b}{i}")
                            for i in range(2)
                        ]
                        for b in range(BL)
                    ]
                    for g in range(NG):
                        wt = w_pool.tile([128, G, 256], BF16, tag="wp", name="wpt")
                        eng = nc.sync if g % 2 == 0 else nc.scalar
                        eng.dma_start(out=wt[:], in_=wp[h * 2 + half, g])
                        for j in range(G):
                            kt = g * G + j
                            for b in range(BL):
                                ak = at_sb[:, kt, b * N : (b + 1) * N]
                                for jt2 in range(2):
                                    nc.tensor.matmul(
                                        pss[b][jt2][:],
                                        wt[:, j, jt2 * 128 : (jt2 + 1) * 128],
                                        ak,
                                        start=(kt == 0),
                                        stop=False,
                                    )
                    for b in range(BL):
                        for jt2 in range(2):
                            jt = half * 2 + jt2
                            for dt in range(2):
                                nc.tensor.matmul(
                                    pss[b][jt2][:],
                                    kt_sbs[b][:, dt, jt * 128 : (jt + 1) * 128],
                                    qt_sbs[b][:, dt, :],
                                    start=False,
                                    stop=(dt == 1),
                                )
                            nc.scalar.activation(
                                e_sbs[b][:, jt, :],
                                pss[b][jt2][:],
                                mybir.ActivationFunctionType.Exp,
                                bias=bias_sb[:, h * 4 + jt : h * 4 + jt + 1],
                            )
                pending = (h, e_sbs, v_sbs)

            attn_out(pending)

    nc.compile()
    return nc


_NC_CACHE = None


def _get_program():
    global _NC_CACHE
    if _NC_CACHE is None:
        _NC_CACHE = _build_program()
    return _NC_CACHE


def _pack_inputs(x, Augementation_embedding, Wq, Wk, Wv, Wp, bp):
    """Host-side relayout: transposes, per-head tiling, bf16 casts."""
    f = np.float32
    bf = ml_dtypes.bfloat16
    x = np.asarray(x, f)
    aug = np.asarray(Augementation_embedding, f)

    # [B, N, D] -> per core [128, KT, 2*N] bf16 (batch pair side by side)
    def act_pack(a):
        t = a.transpose(0, 2, 1).reshape(B, KT, 128, N).transpose(0, 2, 1, 3)
        t = t.reshape(NCORES, BL, 128, KT, N).transpose(0, 2, 3, 1, 4)
        return np.ascontiguousarray(t.reshape(NCORES, 128, KT, BL * N).astype(bf))

    xt = act_pack(x)
    at = act_pack(aug)

    # W.T [D, dout_total] -> [nh, NG, 128, G*256] bf16
    def w_pack(w_t):
        nh = w_t.shape[1] // 256
        t = w_t.reshape(KT, 128, nh, 256).transpose(2, 0, 1, 3)
        t = (
            t.reshape(nh, NG, G, 128, 256)
            .transpose(0, 1, 3, 2, 4)
            .reshape(nh, NG, 128, G * 256)
        )
        return np.ascontiguousarray(t.astype(bf))

    wq_pk = w_pack(np.asarray(Wq, f).T * np.float32(SCALE))
    wk_pk = w_pack(np.asarray(Wk, f).T)
    wv_pk = w_pack(np.asarray(Wv, f).T)
    wp_pk = w_pack(np.asarray(Wp, f).T)  # 32 pseudo-heads of 256 jrows

    bias = np.ascontiguousarray(np.asarray(bp, f).reshape(64, 128).T)  # [128, 64]

    return xt, at, wq_pk, wk_pk, wv_pk, wp_pk, bias


def kernel(x, Augementation_embedding, Wq, Wk, Wv, Wp, bp):
    nc = _get_program()
    xt, at, wq_pk, wk_pk, wv_pk, wp_pk, bias = _pack_inputs(
        x, Augementation_embedding, Wq, Wk, Wv, Wp, bp
    )

    in_maps = []
    for c in range(NCORES):
        in_maps.append(
            {
                "xt": xt[c],
                "at": at[c],
                "wq": wq_pk,
                "wk": wk_pk,
                "wv": wv_pk,
                "wp": wp_pk,
                "bias": bias,
            }
        )

    res = run_bass_kernel_spmd(nc, in_maps, core_ids=list(range(NCORES)))
    outs = [res.results[c]["out"] for c in range(NCORES)]
    return np.concatenate(outs, axis=0).astype(np.float32)


if __name__ == "__main__":
    rng = np.random.default_rng(0)
    ins = {
        "x": rng.standard_normal((B, N, D), dtype=np.float32),
        "Augementation_embedding": rng.standard_normal((B, N, D), dtype=np.float32),
        "Wq": rng.standard_normal((D, D), dtype=np.float32) / np.sqrt(D),
        "Wk": rng.standard_normal((D, D), dtype=np.float32) / np.sqrt(D),
        "Wv": rng.standard_normal((D, D), dtype=np.float32) / np.sqrt(D),
        "Wp": rng.standard_normal((2 * D, D), dtype=np.float32) / np.sqrt(D),
        "bp": (rng.standard_normal(2 * D, dtype=np.float32) * 0.01),
    }
    o = kernel(**ins)
    print("out", o.shape, o.dtype, float(np.abs(o).max()))



# revision 14
# speedup vs baseline: 1.1980x; 1.1980x over previous
"""Trainium2 Bass kernel for nn_AugementationAttention.

Reference computation (per batch b of 16, N=512, D=4096, NH=16, DK=256):
    q = x @ Wq.T, k = x @ Wk.T, v = x @ Wv.T          (per-head dk=256)
    ep = aug @ Wp.T + bp                               (per-head n=512 == 2*dk)
    dist = softmax(q k^T / sqrt(dk) + ep, axis=-1)
    out  = dist @ v                                    -> (b, n, d)

Sharding: data-parallel over batch, 2 batch elements per core on 8 cores.

Per-core kernel structure (single fused pass over heads, batches paired):
  - bf16 everywhere on the matmuls.  fp8 e4m3 fails the 2e-2 absmax gate
    by construction (numpy sweep vs the oracle: V-only 2.2e-2, EP-only
    3.4e-2, QK-only 5.5e-2, all-fp8 1.1e-1 -- and split/compensated fp8
    schemes cost >= 1 bf16-equivalent pass), so the kernel is pinned at
    the 78.6 TFLOP/s bf16 roofline: 180.6 GFLOP/core -> 2294 us of pure
    column streaming.  Everything else keeps the PE at that rate:
  - Warmup: ~72 junk matmuls on a memset tile run during the initial DMA
    shadow so the HAM clock gate reaches K=8/8 (2.4 GHz) before real work
    and never re-throttles.  (Starting real work earlier regresses: the
    DMA rings are cold/slow the first ~10 us, so an earlier start just
    moves the idle into head 0's body.)
  - Head 0 interleaves the Q and K projections per k-tile (8 psum banks),
    halving its x-arrival bandwidth demand; heads 1+ run phase-by-phase
    from resident activations.  Its critical stream is choreographed
    across the three DMA queues: first-needed chunks land in parallel
    (wk g0 in halves so the first K matmuls never wait; bias rides
    scalar, not the gpsimd queue head), the next wq/wk groups are
    prestaged on a dedicated 8-buf pool tag, x streams in k-tile pairs
    in consumption order on the fast SWDGE queue -- except kt21/kt25,
    which ride the HWDGE rings behind the g4 weight chunks: at 2.4 GHz
    Q+K consume x at ~150 GB/s vs ~130 GB/s SWDGE supply, and the
    deficit otherwise surfaces as a ~2.5 us stall near kt26.  The aug
    bulk is gated behind the q/k copies so it cannot starve the weight
    rings.  (Offloading x EARLY in the HWDGE queues regresses: HBM
    arbiter contention in the first 30 us slows the SWDGE stream.)
  - Scores stay in the transposed [key, query] layout; softmax
    normalization over the partition (key) axis uses a ones-column
    appended to V; bp folds into ScalarE's exp bias.  exp(S), V, and
    q/k are stored bf16 (q/k quantization adds ~2e-3 absmax, and bf16
    LDWEIGHTS on the QK^T stationaries stay hidden).  The A@V stage of
    head h is deferred past head h+1's Q projection so the PE never
    waits on ScalarE's exp.  Output stores are bf16 (host upcasts),
    halving store traffic; per-qt stores round-robin all three rings.

Measured: 2.358 ms HW exec at 2.4 GHz (prev best 2.360, original
baseline 2.399), rel err 7.5e-3 vs the 2e-2 gate.  Loss budget over the
2294 us stream floor: 7.3 us fixed preamble, ~3.8 us warmup, ~31 us
per-matmul issue overhead (~2.5 ns x 13k, irreducible), ~5 us semaphore
sprinkles, ~16 us tail of which ~10 us is fixed framework epilogue
(drain/barrier rounds after the last store lands).  Runs land at 2.82 ms
when the chip drops to 2.0 GHz (P0 power state) -- environmental, not
kernel-dependent (at that clock the trace shows a single 0.7 us gap over
the whole body).
"""

import sys

sys.path.insert(0, "/opt/trn_rl_repo")

import numpy as np
import ml_dtypes

import concourse.bacc as bacc
import concourse.mybir as mybir
import concourse.tile as tile
from concourse.bass_utils import run_bass_kernel_spmd

F32 = mybir.dt.float32
F32R = mybir.dt.float32r
BF16 = mybir.dt.bfloat16

B, N, D, NH, DK = 16, 512, 4096, 16, 256
NCORES = 8
BL = B // NCORES  # batch elements per core
NB = BL * N  # 1024: both batch elements side by side on the free dim
KT = D // 128  # 32 k-tiles of the contraction dim
G = 4  # k-tiles per weight DMA chunk
NG = KT // G  # chunks per head per projection
SCALE = 1.0 / np.sqrt(DK)


def _build_program():
    nc = bacc.Bacc(
        "TRN2",
        target_bir_lowering=False,
        debug=False,
        enable_asserts=False,
        num_devices=NCORES,
    )

    xt = nc.dram_tensor("xt", [128, KT, NB], BF16, kind="ExternalInput")
    at = nc.dram_tensor("at", [128, KT, NB], BF16, kind="ExternalInput")
    # w*[h, g, p, G*256]: per chunk g, k-tile j in chunk, 256 dout cols
    wq = nc.dram_tensor("wq", [NH, NG, 128, G * 256], BF16, kind="ExternalInput")
    wk = nc.dram_tensor("wk", [NH, NG, 128, G * 256], BF16, kind="ExternalInput")
    wv = nc.dram_tensor("wv", [NH, NG, 128, G * 256], BF16, kind="ExternalInput")
    # wp packed as 32 pseudo-heads of 256 jrow cols: index h*2 + jt_half
    wp = nc.dram_tensor("wp", [2 * NH, NG, 128, G * 256], BF16, kind="ExternalInput")
    bias = nc.dram_tensor("bias", [128, 64], F32, kind="ExternalInput")
    # bf16 output: halves the final store drain (the kernel tail) and the
    # mid-run store traffic; host upcasts.  Costs ~2e-3 absmax rel err.
    out = nc.dram_tensor("out", [BL, N, D], BF16, kind="ExternalOutput")

    with tile.TileContext(nc) as tc:
        with (
            tc.tile_pool(name="const", bufs=1) as const_pool,
            tc.tile_pool(name="acts", bufs=1) as act_pool,
            tc.tile_pool(name="wgt", bufs=4) as w_pool,
            tc.tile_pool(name="qk", bufs=2) as qk_pool,
            tc.tile_pool(name="vv", bufs=2) as v_pool,
            tc.tile_pool(name="ee", bufs=2) as e_pool,
            tc.tile_pool(name="oo", bufs=3) as o_pool,
            tc.tile_pool(name="rr", bufs=8) as r_pool,
            tc.tile_pool(name="psp", bufs=4, space="PSUM") as ps_a,
            tc.tile_pool(name="pss", bufs=4, space="PSUM") as ps_b,
        ):
            bias_sb = const_pool.tile([128, 64], F32)

            # HAM warmup: keep the PE busy (~3.5us) under the initial DMA
            # shadow so the clock gate is at 8/8 when real matmuls start.
            junk_sb = const_pool.tile([128, 128], BF16, name="junk_sb")
            nc.vector.memset(junk_sb[:], 0.0)
            warm_ps = ps_b.tile([128, N], F32, tag="ss", name="warm_ps")
            for _ in range(72):
                nc.tensor.matmul(
                    warm_ps[:, 0:128],
                    junk_sb[:, 0:128],
                    junk_sb[:, 0:128],
                    start=True,
                    stop=True,
                )

            # resident activations, streamed in consumption order on the two
            # SWDGE rings; the sync/scalar HWDGE rings carry only weights so
            # the first-head critical path is never queued behind bulk.
            xt_sb = act_pool.tile([128, KT, NB], BF16, tag="xt", name="xt_sb")
            at_sb = act_pool.tile([128, KT, NB], BF16, tag="at", name="at_sb")
            # First-needed chunks land in parallel on all three queues; the
            # next three wq/wk groups ride the fast SWDGE queue ahead of the
            # x bulk (dedicated 8-buf tag -- triggers never stall on pool
            # recycle), leaving the slow HWDGE rings only g4..g7 of head 0.
            # (A finer-grained first wave was tried and regressed: the DMA
            # rings are cold (~26 GB/s) for the first ~10us, so starting the
            # stream earlier just moves the idle into heads 0's body.)
            w0_pre = []
            for g in range(4):
                wtq0 = w_pool.tile([128, G, 256], BF16, tag="w0", bufs=8, name="wq0p")
                wtk0 = w_pool.tile([128, G, 256], BF16, tag="w0", bufs=8, name="wk0p")
                if g == 0:
                    # three queues in parallel for the three first-needed chunks
                    # (wk g0 split in halves: its kt0-1 stationaries gate the
                    # K matmuls right after the first Q matmuls)
                    nc.sync.dma_start(out=wtq0[:], in_=wq[0, g])
                    nc.scalar.dma_start(out=xt_sb[:, 0:2, :], in_=xt[:, 0:2, :])
                    nc.gpsimd.dma_start(out=wtk0[:, 0:2, :], in_=wk[0, 0][:, 0:512])
                    nc.gpsimd.dma_start(out=wtk0[:, 2:4, :], in_=wk[0, 0][:, 512:1024])
                    nc.scalar.dma_start(out=bias_sb[:], in_=bias[:])
                elif g == 1:
                    # the HWDGE rings are free again by now; keep the SWDGE
                    # chain short so g2's x pairs aren't queued behind these
                    nc.sync.dma_start(out=wtq0[:], in_=wq[0, g])
                    nc.scalar.dma_start(out=wtk0[:], in_=wk[0, g])
                else:
                    nc.gpsimd.dma_start(out=wtq0[:], in_=wq[0, g])
                    nc.gpsimd.dma_start(out=wtk0[:], in_=wk[0, g])
                # x pairs for this g-group follow its weights in the FIFO
                # (kt0-1 already went out on the scalar ring above)
                if g > 0:
                    sl = slice(4 * g, 4 * g + 2)
                    nc.gpsimd.dma_start(out=xt_sb[:, sl, :], in_=xt[:, sl, :])
                sl = slice(4 * g + 2, 4 * g + 4)
                nc.gpsimd.dma_start(out=xt_sb[:, sl, :], in_=xt[:, sl, :])
                w0_pre.append((wtq0, wtk0))
            # rest of x in kt pairs, consumption order; kt21/kt25 are
            # carried by the HWDGE rings behind head-0's weight chunks (see
            # proj_qk_head0) to relieve the SWDGE queue before kt26.
            for i in range(8, 16):
                k0 = 2 * i
                if k0 in (20, 24):
                    nc.gpsimd.dma_start(
                        out=xt_sb[:, k0 : k0 + 1, :], in_=xt[:, k0 : k0 + 1, :]
                    )
                else:
                    sl = slice(k0, k0 + 2)
                    nc.gpsimd.dma_start(out=xt_sb[:, sl, :], in_=xt[:, sl, :])

            def proj_qk(w_dram, h, name):
                """Q^T/K^T projection for both batches: psum[b][dt] [128, N]."""
                ps = [
                    [
                        ps_a.tile([128, N], F32, tag="pp", name=f"ps{name}{b}{i}")
                        for i in range(2)
                    ]
                    for b in range(BL)
                ]
                for g in range(NG):
                    wt = w_pool.tile([128, G, 256], BF16, tag="wqk", name=f"w{name}")
                    eng = nc.sync if g % 2 == 0 else nc.scalar
                    eng.dma_start(out=wt[:], in_=w_dram[h, g])
                    for j in range(G):
                        kt = g * G + j
                        st, sp = kt == 0, kt == KT - 1
                        for b in range(BL):
                            xk = xt_sb[:, kt, b * N : (b + 1) * N]
                            for dt in range(2):
                                nc.tensor.matmul(
                                    ps[b][dt][:],
                                    wt[:, j, dt * 128 : (dt + 1) * 128],
                                    xk,
                                    start=st,
                                    stop=sp,
                                )
                sbs = []
                for b in range(BL):
                    sb = qk_pool.tile(
                        [128, 2, N], BF16, tag=f"{name}t", name=f"{name}t_sb{b}"
                    )
                    for dt in range(2):
                        nc.vector.tensor_copy(sb[:, dt, :], ps[b][dt][:])
                    sbs.append(sb)
                return sbs

            def proj_qk_head0():
                """Head 0 only: Q and K interleaved per k-tile, so the x
                stream is consumed at half the bandwidth of a serial Q pass
                (x is still arriving from HBM while this runs)."""
                psq = [
                    [
                        ps_a.tile([128, N], F32, tag="pp", name=f"psq0{b}{i}")
                        for i in range(2)
                    ]
                    for b in range(BL)
                ]
                psk = [
                    [
                        ps_b.tile([128, N], F32, tag="ss", name=f"psk0{b}{i}")
                        for i in range(2)
                    ]
                    for b in range(BL)
                ]
                for g in range(NG):
                    # one weight stream per HWDGE ring: under DMA-arbiter
                    # pressure from the x bulk each ring sustains only
                    # ~40 GB/s, so wq and wk must not share a ring here
                    # (g<4 was prefetched above).
                    if g < 4:
                        wtq, wtk = w0_pre[g]
                    else:
                        wtq = w_pool.tile([128, G, 256], BF16, tag="wqk", name="wq0")
                        wtk = w_pool.tile([128, G, 256], BF16, tag="wqk", name="wk0")
                        nc.sync.dma_start(out=wtq[:], in_=wq[0, g])
                        nc.scalar.dma_start(out=wtk[:], in_=wk[0, g])
                        if g == 4:
                            # emitted BEFORE their consuming matmuls (kt21 is
                            # consumed in the g=5 block) so the dependency
                            # edge is real; ring-wise they sit behind the g4
                            # weight chunks, late enough to dodge the early
                            # HBM-arbiter crunch
                            nc.sync.dma_start(
                                out=xt_sb[:, 21:22, :], in_=xt[:, 21:22, :]
                            )
                            nc.scalar.dma_start(
                                out=xt_sb[:, 25:26, :], in_=xt[:, 25:26, :]
                            )
                    for j in range(G):
                        kt = g * G + j
                        st, sp = kt == 0, kt == KT - 1
                        for b in range(BL):
                            xk = xt_sb[:, kt, b * N : (b + 1) * N]
                            for dt in range(2):
                                nc.tensor.matmul(
                                    psq[b][dt][:],
                                    wtq[:, j, dt * 128 : (dt + 1) * 128],
                                    xk,
                                    start=st,
                                    stop=sp,
                                )
                                nc.tensor.matmul(
                                    psk[b][dt][:],
                                    wtk[:, j, dt * 128 : (dt + 1) * 128],
                                    xk,
                                    start=st,
                                    stop=sp,
                                )
                qsbs, ksbs = [], []
                for b in range(BL):
                    qsb = qk_pool.tile([128, 2, N], BF16, tag="qt", name=f"qt_sb{b}")
                    ksb = qk_pool.tile([128, 2, N], BF16, tag="kt", name=f"kt_sb{b}")
                    for dt in range(2):
                        nc.vector.tensor_copy(qsb[:, dt, :], psq[b][dt][:])
                        nc.vector.tensor_copy(ksb[:, dt, :], psk[b][dt][:])
                    qsbs.append(qsb)
                    ksbs.append(ksb)
                # Gate the aug bulk behind the q copies with a REAL data
                # dependency (a memset has none and the Tile scheduler
                # hoists it): if aug starts any earlier, the SWDGE queue
                # races ahead and starves the HWDGE weight rings mid-phase.
                # The garbage cell is overwritten by the chunk-0 DMA (WAW).
                nc.vector.tensor_copy(at_sb[:, 0:1, 0:1], qsbs[0][:, 0:1, 0:1])
                for i in range(8):  # aug, needed from the first S phase
                    sl = slice(4 * i, 4 * i + 4)
                    nc.gpsimd.dma_start(out=at_sb[:, sl, :], in_=at[:, sl, :])
                return qsbs, ksbs

            def proj_v(h):
                """V projection (n-major) for both batches, ones col appended."""
                ps = [
                    [
                        ps_a.tile([128, N], F32, tag="pp", name=f"psv{b}{i}")
                        for i in range(2)
                    ]
                    for b in range(BL)
                ]
                for g in range(NG):
                    wt = w_pool.tile([128, G, 256], BF16, tag="wqk", name="wv")
                    eng = nc.sync if g % 2 == 0 else nc.scalar
                    eng.dma_start(out=wt[:], in_=wv[h, g])
                    for j in range(G):
                        kt = g * G + j
                        wvj = wt[:, j, :]
                        for b in range(BL):
                            for nt in range(4):
                                nc.tensor.matmul(
                                    ps[b][nt // 2][
                                        :, (nt % 2) * 256 : (nt % 2 + 1) * 256
                                    ],
                                    xt_sb[
                                        :, kt, b * N + nt * 128 : b * N + (nt + 1) * 128
                                    ],
                                    wvj,
                                    start=(kt == 0 and nt % 2 == 0),
                                    stop=(kt == KT - 1 and nt % 2 == 1),
                                )
                v_sbs = []
                for b in range(BL):
                    v_sb = v_pool.tile([128, 4, 264], BF16, tag="v", name=f"v_sb{b}")
                    for nt in range(4):
                        nc.vector.tensor_copy(
                            v_sb[:, nt, 0:256],
                            ps[b][nt // 2][:, (nt % 2) * 256 : (nt % 2 + 1) * 256],
                        )
                    nc.vector.memset(v_sb[:, :, 256:257], 1.0)
                    nc.vector.memset(v_sb[:, :, 257:258], 0.0)
                    v_sbs.append(v_sb)
                return v_sbs

            def attn_out(state):
                """Deferred A@V + normalize + store for the previous head."""
                if state is None:
                    return
                h, e_sbs, v_sbs = state
                for b in range(BL):
                    e_sb, v_sb = e_sbs[b], v_sbs[b]
                    ot_sb = o_pool.tile([128, 4, 256], BF16, tag="ot", name="ot_sb")
                    out_v = out[b].rearrange("(qt p) d -> p qt d", p=128)
                    for qt in range(4):
                        pso = ps_b.tile([128, 258], F32, tag="ss", name="pso")
                        for jt in range(4):
                            nc.tensor.matmul(
                                pso[:],
                                e_sb[:, jt, qt * 128 : (qt + 1) * 128],
                                v_sb[:, jt, 0:258],
                                start=(jt == 0),
                                stop=(jt == 3),
                            )
                        r = r_pool.tile([128, 1], F32, tag="r", name="r")
                        nc.vector.reciprocal(r[:], pso[:, 256:257])
                        nc.vector.tensor_scalar_mul(
                            ot_sb[:, qt, :], pso[:, 0:256], r[:]
                        )
                        # small per-qt stores, round-robin over all three rings
                        eng = (nc.sync, nc.scalar, nc.gpsimd)[(h * 8 + b * 4 + qt) % 3]
                        eng.dma_start(
                            out=out_v[:, qt, h * 256 : (h + 1) * 256],
                            in_=ot_sb[:, qt, :],
                        )

            pending = None
            for h in range(NH):
                if h == 0:
                    qt_sbs, kt_sbs = proj_qk_head0()
                else:
                    qt_sbs = proj_qk(wq, h, "q")
                    # previous head's A@V runs here: ScalarE has long finished
                    # the previous exp, and E/V tiles free up before this
                    # head's own S phase needs their pool slots.
                    attn_out(pending)
                    pending = None
                    kt_sbs = proj_qk(wk, h, "k")
                v_sbs = proj_v(h)

                # S^T[j, q] in two jt-halves: Wp-projection + QK^T, then exp
                # bf16 exp storage: the A@V stationary loads then run at
                # bf16 LDWEIGHTS speed (107ns, hidden) instead of fp32
                # (190ns, partially exposed)
                e_sbs = [
                    e_pool.tile([128, 4, N], BF16, tag="e", name=f"e_sb{b}")
                    for b in range(BL)
                ]
                for half in range(2):
                    pool = ps_b if half == 0 else ps_a
                    ptag = "ss" if half == 0 else "pp"
                    pss = [
                        [
                            pool.tile([128, N], F32, tag=ptag, name=f"pss{b}{i}")
                            for i in range(2)
                        ]
                        for b in range(BL)
                    ]
                    for g in range(NG):
                        wt = w_pool.tile([128, G, 256], BF16, tag="wp", name="wpt")
                        eng = nc.sync if g % 2 == 0 else nc.scalar
                        eng.dma_start(out=wt[:], in_=wp[h * 2 + half, g])
                        for j in range(G):
                            kt = g * G + j
                            for b in range(BL):
                                ak = at_sb[:, kt, b * N : (b + 1) * N]
                                for jt2 in range(2):
                                    nc.tensor.matmul(
                                        pss[b][jt2][:],
                                        wt[:, j, jt2 * 128 : (jt2 + 1) * 128],
                                        ak,
                                        start=(kt == 0),
                                        stop=False,
                                    )
                    for b in range(BL):
                        for jt2 in range(2):
                            jt = half * 2 + jt2
                            for dt in range(2):
                                nc.tensor.matmul(
                                    pss[b][jt2][:],
                                    kt_sbs[b][:, dt, jt * 128 : (jt + 1) * 128],
                                    qt_sbs[b][:, dt, :],
                                    start=False,
                                    stop=(dt == 1),
                                )
                            nc.scalar.activation(
                                e_sbs[b][:, jt, :],
                                pss[b][jt2][:],
                                mybir.ActivationFunctionType.Exp,
                                bias=bias_sb[:, h * 4 + jt : h * 4 + jt + 1],
                            )
                pending = (h, e_sbs, v_sbs)

            attn_out(pending)

    nc.compile()
    return nc


_NC_CACHE = None


def _get_program():
    global _NC_CACHE
    if _NC_CACHE is None:
        _NC_CACHE = _build_program()
    return _NC_CACHE


def _pack_inputs(x, Augementation_embedding, Wq, Wk, Wv, Wp, bp):
    """Host-side relayout: transposes, per-head tiling, bf16 casts."""
    f = np.float32
    bf = ml_dtypes.bfloat16
    x = np.asarray(x, f)
    aug = np.asarray(Augementation_embedding, f)

    # [B, N, D] -> per core [128, KT, 2*N] bf16 (batch pair side by side)
    def act_pack(a):
        t = a.transpose(0, 2, 1).reshape(B, KT, 128, N).transpose(0, 2, 1, 3)
        t = t.reshape(NCORES, BL, 128, KT, N).transpose(0, 2, 3, 1, 4)
        return np.ascontiguousarray(t.reshape(NCORES, 128, KT, BL * N).astype(bf))

    xt = act_pack(x)
    at = act_pack(aug)

    # W.T [D, dout_total] -> [nh, NG, 128, G*256] bf16
    def w_pack(w_t):
        nh = w_t.shape[1] // 256
        t = w_t.reshape(KT, 128, nh, 256).transpose(2, 0, 1, 3)
        t = (
            t.reshape(nh, NG, G, 128, 256)
            .transpose(0, 1, 3, 2, 4)
            .reshape(nh, NG, 128, G * 256)
        )
        return np.ascontiguousarray(t.astype(bf))

    wq_pk = w_pack(np.asarray(Wq, f).T * np.float32(SCALE))
    wk_pk = w_pack(np.asarray(Wk, f).T)
    wv_pk = w_pack(np.asarray(Wv, f).T)
    wp_pk = w_pack(np.asarray(Wp, f).T)  # 32 pseudo-heads of 256 jrows

    bias = np.ascontiguousarray(np.asarray(bp, f).reshape(64, 128).T)  # [128, 64]

    return xt, at, wq_pk, wk_pk, wv_pk, wp_pk, bias


def kernel(x, Augementation_embedding, Wq, Wk, Wv, Wp, bp):
    nc = _get_program()
    xt, at, wq_pk, wk_pk, wv_pk, wp_pk, bias = _pack_inputs(
        x, Augementation_embedding, Wq, Wk, Wv, Wp, bp
    )

    in_maps = []
    for c in range(NCORES):
        in_maps.append(
            {
                "xt": xt[c],
                "at": at[c],
                "wq": wq_pk,
                "wk": wk_pk,
                "wv": wv_pk,
                "wp": wp_pk,
                "bias": bias,
            }
        )

    res = run_bass_kernel_spmd(nc, in_maps, core_ids=list(range(NCORES)))
    outs = [res.results[c]["out"] for c in range(NCORES)]
    return np.concatenate(outs, axis=0).astype(np.float32)


if __name__ == "__main__":
    rng = np.random.default_rng(0)
    ins = {
        "x": rng.standard_normal((B, N, D), dtype=np.float32),
        "Augementation_embedding": rng.standard_normal((B, N, D), dtype=np.float32),
        "Wq": rng.standard_normal((D, D), dtype=np.float32) / np.sqrt(D),
        "Wk": rng.standard_normal((D, D), dtype=np.float32) / np.sqrt(D),
        "Wv": rng.standard_normal((D, D), dtype=np.float32) / np.sqrt(D),
        "Wp": rng.standard_normal((2 * D, D), dtype=np.float32) / np.sqrt(D),
        "bp": (rng.standard_normal(2 * D, dtype=np.float32) * 0.01),
    }
    o = kernel(**ins)
    print("out", o.shape, o.dtype, float(np.abs(o).max()))

